# revision 25
# baseline (speedup 1.0000x reference)
"""Trainium2 Bass kernel: MultiHeadSelfAttention + residual + LayerNorm.

Problem: x[2,2048,1024], H=16 heads, in_proj -> attention -> out_proj ->
residual -> LayerNorm; also returns mean-over-heads attention weights.

Sharding (8 cores, no collectives):
  core c handles batch b=c//4 and query rows [sh*512,(sh+1)*512), sh=c%4.
  K/V projections are computed redundantly per batch group (4 cores share a
  batch, each computes the full K/V for its batch: +8.6 GFLOP/core, which is
  cheaper than any cross-core exchange at trn2 collective rates).

Layouts (all chosen so every matmul contracts over the partition dim):
  qkv^T:  Q^T[dq, rows], K^T[dk, rows] via W^T-stationary matmuls
  V:      [rows, dv]     via x^T-stationary matmuls
  scores^T[k, q] = (K^T tile).T @ Q^T  (head pairs packed in PE row groups)
  exp on ACT (PSUM->SBUF, fp16), denominator fused into the ctx matmul by
  augmenting V with a ones-column block: out rows = [ctx^T | dup(denom)].
  ctx^T scaled by 1/denom, out_proj with ctx^T-stationary -> straight [row, d]
  layout so residual+LayerNorm use bn_stats along the free dim.
  attn-weight mean accumulated as sum_h e_h * (1/denom_h) in fp16; the /H and
  the final [k,q]->[q,k] transpose happen on the host.
"""

import sys

for _p in ("/opt/trn_rl_repo",):
    if _p not in sys.path:
        sys.path.insert(0, _p)

import numpy as np

import concourse.bass as bass
from concourse import bacc
import concourse.mybir as mybir
import concourse.tile as tile
from concourse.bass_utils import run_bass_kernel_spmd
F32 = mybir.dt.float32
F32R = mybir.dt.float32r
F16 = mybir.dt.float16
AF = mybir.ActivationFunctionType

NCORES = 8
B, S, D = 2, 2048, 1024
H, HD = 16, 64
P = 128
R = S // 4            # 512 query rows per core
NKT = S // P          # 16 k tiles
NDT = D // P          # 8 d tiles
NHP = H // 2          # 8 head pairs
LN_EPS = 1e-5

# V sbuf free-dim layout: per k-tile, 16 heads of [V_h(64) | ones(1)] = 1040
VHW = 65                    # per-head width (64 V cols + 1 ones col)
VKT = H * VHW               # 1040 cols per k-tile group
VCOLS = NKT * VKT


def _ap(t, extra_offset, ap_dims):
    return bass.AP(tensor=t.tensor, offset=t.offset + extra_offset, ap=ap_dims)


def build_program(has_qkv_bias, has_out_bias, has_gamma, has_beta):
    nc = bacc.Bacc("TRN2", target_bir_lowering=False, debug=False)

    xT = nc.dram_tensor("xT", [D, S], F32R, kind="ExternalInput").ap()
    xQT = nc.dram_tensor("xQT", [D, R], F32R, kind="ExternalInput").ap()
    xrow = nc.dram_tensor("xrow", [R, D], F32, kind="ExternalInput").ap()
    wqT = nc.dram_tensor("wqT", [D, D], F32R, kind="ExternalInput").ap()
    wkT = nc.dram_tensor("wkT", [D, D], F32R, kind="ExternalInput").ap()
    wvT = nc.dram_tensor("wvT", [D, D], F32R, kind="ExternalInput").ap()
    woT = nc.dram_tensor("woT", [D, D], F32R, kind="ExternalInput").ap()
    # bias layout: [bq/8, bk, bv, bo] (4*D)
    bvec = nc.dram_tensor("bvec", [4 * D], F32R, kind="ExternalInput").ap()
    gamma = nc.dram_tensor("gamma", [D], F32, kind="ExternalInput").ap()
    beta = nc.dram_tensor("beta", [D], F32, kind="ExternalInput").ap()

    out_c = nc.dram_tensor("out_c", [R, D], F32, kind="ExternalOutput").ap()
    wmeanT = nc.dram_tensor("wmeanT", [S, R], F16, kind="ExternalOutput").ap()

    kT_dram = nc.dram_tensor("kT_scratch", [NDT, P, S], F32R)

    with tile.TileContext(nc) as tc:
        with tc.tile_pool(name="persist", bufs=1) as persist:
            v_sb = persist.tile([P, VCOLS], F16)
            q_sb = persist.tile([P, NDT, R], F32R)
            ctx_sb = persist.tile([P, NHP, R], F32R)
            wacc = persist.tile([P, NKT, R], F16)
            wacc2 = persist.tile([P, NKT, R], F16)
            nc.vector.memset(
                _ap(v_sb, 64, [[VCOLS, P], [VHW, NKT * H], [1, 1]]), 1.0)
            b_sb = ones_col = ones_row = None
            if has_qkv_bias or has_out_bias:
                b_sb = persist.tile([1, 4 * D], F32R)
                ones_col = persist.tile([1, P], F32R)
                ones_row = persist.tile([1, 512], F32R)
                nc.sync.dma_start(out=b_sb[:], in_=_ap(bvec, 0, [[0, 1], [1, 4 * D]]))
                nc.vector.memset(ones_col[:], 1.0)
                nc.vector.memset(ones_row[:], 1.0)

            # ---------------- Phase 1: projections ----------------
            # Q first (smallest inputs -> earliest PE start)
            with tc.tile_pool(name="projq_sbuf", bufs=1) as pq, \
                 tc.tile_pool(name="projq_w", bufs=12) as qwpool, \
                 tc.tile_pool(name="projq_ps", bufs=4, space="PSUM") as qps:
                xQT_sb = pq.tile([P, NDT, R], F32R)
                for dt in range(NDT):
                    nc.sync.dma_start(out=xQT_sb[:, dt, :], in_=xQT[dt * P:(dt + 1) * P, :])
                for ft in range(NDT):
                    qw = []
                    for dt in range(NDT):
                        qwt = qwpool.tile([P, P], F32R, tag="kw")
                        nc.sync.dma_start(
                            out=qwt[:],
                            in_=wqT[dt * P:(dt + 1) * P, ft * P:(ft + 1) * P])
                        qw.append(qwt)
                    ps = qps.tile([P, 512], F32, tag="ps")
                    for dt in range(NDT):
                        nc.tensor.matmul(
                            ps[:], qw[dt][:], xQT_sb[:, dt, :],
                            start=(dt == 0),
                            stop=(dt == NDT - 1 and not has_qkv_bias))
                    if has_qkv_bias:
                        nc.tensor.matmul(
                            ps[:], b_sb[:, ft * P:(ft + 1) * P], ones_row[:],
                            start=False, stop=True)
                    nc.vector.tensor_copy(q_sb[:, ft, :], ps[:])

            # K^T and V projections, x^T streamed in sequence-quarters
            with tc.tile_pool(name="proj_sbuf", bufs=1) as pj, \
                 tc.tile_pool(name="projx", bufs=2) as pxpool, \
                 tc.tile_pool(name="wtiles", bufs=3) as wpool, \
                 tc.tile_pool(name="proj_ps", bufs=4, space="PSUM") as pps:
                kw_sb = pj.tile([P, NDT, D], F32R)
                wv_sb = pj.tile([P, NDT, D], F32R)
                for dt in range(NDT):
                    nc.sync.dma_start(out=kw_sb[:, dt, :], in_=wkT[dt * P:(dt + 1) * P, :])
                    nc.sync.dma_start(out=wv_sb[:, dt, :], in_=wvT[dt * P:(dt + 1) * P, :])
                for qp in range(2):
                    xhs = []
                    for qq in range(2):
                        quarter = qp * 2 + qq
                        xh = pxpool.tile([P, NDT, 512], F32R, tag="xTh",
                                         name=f"xh{quarter}")
                        for dt in range(NDT):
                            nc.sync.dma_start(
                                out=xh[:, dt, :],
                                in_=xT[dt * P:(dt + 1) * P,
                                       quarter * 512:quarter * 512 + 512])
                        xhs.append(xh)

                    # K^T -> DRAM scratch; stationary kw reused across the
                    # two resident quarters
                    for ft in range(NDT):
                        pss = [pps.tile([P, 512], F32, tag="ps",
                                        name=f"kps{ft}_{qq}") for qq in range(2)]
                        for dt in range(NDT):
                            for qq in range(2):
                                nc.tensor.matmul(
                                    pss[qq][:], kw_sb[:, dt, ft * P:(ft + 1) * P],
                                    xhs[qq][:, dt, :],
                                    start=(dt == 0),
                                    stop=(dt == NDT - 1 and not has_qkv_bias))
                        for qq in range(2):
                            if has_qkv_bias:
                                nc.tensor.matmul(
                                    pss[qq][:], b_sb[:, D + ft * P:D + (ft + 1) * P],
                                    ones_row[:], start=False, stop=True)
                            kst = wpool.tile([P, 512], F32R, tag="kst")
                            nc.vector.tensor_copy(kst[:], pss[qq][:])
                            h0 = (qp * 2 + qq) * 512
                            nc.gpsimd.dma_start(
                                out=kT_dram[ft, :, h0:h0 + 512], in_=kst[:])

                    # V -> SBUF fp16; stationary xh tile reused across fc
                    for qq in range(2):
                        quarter = qp * 2 + qq
                        for rtq in range(4):
                            rt = quarter * 4 + rtq
                            pss = [pps.tile([P, 512], F32, tag="ps",
                                            name=f"vps{rt}_{fc}") for fc in range(2)]
                            for dt in range(NDT):
                                for fc in range(2):
                                    nc.tensor.matmul(
                                        pss[fc][:], xhs[qq][:, dt, rtq * P:(rtq + 1) * P],
                                        wv_sb[:, dt, fc * 512:(fc + 1) * 512],
                                        start=(dt == 0),
                                        stop=(dt == NDT - 1 and not has_qkv_bias))
                            for fc in range(2):
                                if has_qkv_bias:
                                    nc.tensor.matmul(
                                        pss[fc][:], ones_col[:],
                                        b_sb[:, 2 * D + fc * 512:2 * D + (fc + 1) * 512],
                                        start=False, stop=True)
                                nc.vector.tensor_copy(
                                    _ap(v_sb, rt * VKT + fc * 8 * VHW,
                                        [[VCOLS, P], [VHW, 8], [1, 64]]),
                                    pss[fc][:])

            # ---------------- Phase 2: attention ----------------
            chunks = [(0, 3), (3, 3), (6, 3), (9, 3), (12, 3), (15, 1)]
            with tc.tile_pool(name="attn_sbuf", bufs=4) as ap_, \
                 tc.tile_pool(name="kp_pool", bufs=2) as kpool, \
                 tc.tile_pool(name="r_pool", bufs=2) as rpool, \
                 tc.tile_pool(name="wt_pool", bufs=2) as wtpool, \
                 tc.tile_pool(name="score_ps", bufs=2, space="PSUM") as sps, \
                 tc.tile_pool(name="ctx_ps", bufs=2, space="PSUM") as cps:
                for hp in range(NHP):
                    kp = kpool.tile([P, S], F32R, tag="kp")
                    nc.sync.dma_start(out=kp[:], in_=kT_dram[hp])
                    ctx_ps = {}
                    e_sb = {}
                    for h in (2 * hp, 2 * hp + 1):
                        ctx_ps[h] = cps.tile([65, R], F32, tag="ctx", name=f"ctx{h}")
                        e_sb[h] = ap_.tile([P, NKT, R], F16, tag="e", name=f"e{h}")
                    for (k0, klen) in chunks:
                        for h in (2 * hp, 2 * hp + 1):
                            hb = (h % 2) * 64
                            ps = sps.tile([P, 3, R], F32, tag="sc")
                            for i in range(klen):
                                kt = k0 + i
                                nc.tensor.matmul(
                                    ps[:, i, :],
                                    kp[hb:hb + 64, kt * P:(kt + 1) * P],
                                    q_sb[hb:hb + 64, hp, :],
                                    start=True, stop=True)
                            nc.scalar.activation(
                                e_sb[h][:, k0:k0 + klen, :], ps[:, 0:klen, :], AF.Exp)
                            # ctx accumulation: lhsT = [V_h(64) | ones(1)]
                            # contiguous -> psum rows 0:64 ctx, row 64 denom
                            for i in range(klen):
                                kt = k0 + i
                                vstart = kt * VKT + h * VHW
                                nc.tensor.matmul(
                                    ctx_ps[h][:], v_sb[:, vstart:vstart + VHW],
                                    e_sb[h][:, kt, :],
                                    start=(kt == 0), stop=(kt == NKT - 1))
                    for h in (2 * hp, 2 * hp + 1):
                        # ctx rows 0:64, denominator in row 64
                        r1 = rpool.tile([P, R], F16, tag="r1")
                        r128 = rpool.tile([P, R], F16, tag="r128")
                        with nc.allow_low_precision(reason="softmax recip to fp16"):
                            nc.vector.reciprocal(r1[64:65, :], ctx_ps[h][64:65, :])
                        nc.sync.dma_start(out=r1[0:1, :], in_=r1[64:65, :])
                        nc.gpsimd.partition_broadcast(r128[:], r1[0:1, :])
                        if h % 2 == 0:
                            nc.vector.tensor_mul(
                                ctx_sb[0:64, hp, :], ctx_ps[h][0:64, :], r128[0:64, :])
                        else:
                            ctmp = rpool.tile([P, R], F32R, tag="ctmp")
                            nc.vector.tensor_mul(
                                ctmp[0:64, :], ctx_ps[h][0:64, :], r128[0:64, :])
                            nc.sync.dma_start(
                                out=ctx_sb[64:128, hp, :], in_=ctmp[0:64, :])
                        # attn-weight mean accumulation (fp16):
                        #   wacc += e_h * r128   (first head: plain write)
                        acc = wacc if h % 2 == 0 else wacc2
                        for half in range(4):
                            sl = slice(half * 4, half * 4 + 4)
                            rb = _ap(r128, 0, [[R, P], [0, 4], [1, R]])
                            if h < 2:
                                nc.vector.tensor_mul(
                                    acc[:, sl, :], e_sb[h][:, sl, :], rb)
                            else:
                                wt = wtpool.tile([P, 4, R], F16, tag="wt")
                                nc.vector.tensor_mul(wt[:], e_sb[h][:, sl, :], rb)
                                nc.vector.tensor_add(
                                    acc[:, sl, :], acc[:, sl, :], wt[:])

            for half in range(2):
                sl = slice(half * 8, half * 8 + 8)
                nc.vector.tensor_add(wacc[:, sl, :], wacc[:, sl, :],
                                     wacc2[:, sl, :])
            nc.sync.dma_start(
                out=wmeanT.rearrange("(kt p) q -> p kt q", p=P), in_=wacc[:])

            # ---------------- Phase 3: out_proj + residual + LayerNorm ------
            with tc.tile_pool(name="tail_sbuf", bufs=1) as tl, \
                 tc.tile_pool(name="tail_y", bufs=4) as ty, \
                 tc.tile_pool(name="tail_wo", bufs=4) as two, \
                 tc.tile_pool(name="tail_tmp", bufs=3) as tt, \
                 tc.tile_pool(name="tail_ps", bufs=1, space="PSUM") as tps:
                xr_sb = tl.tile([P, 4, D], F32)
                eps_sb = tl.tile([P, 1], F32)
                nc.vector.memset(eps_sb[:], LN_EPS)
                for rt in range(4):
                    nc.sync.dma_start(
                        out=xr_sb[:, rt, :], in_=xrow[rt * P:(rt + 1) * P, :])
                if has_gamma:
                    g_sb = tl.tile([P, D], F32)
                    nc.sync.dma_start(out=g_sb[:], in_=_ap(gamma, 0, [[0, P], [1, D]]))
                if has_beta:
                    be_sb = tl.tile([P, D], F32)
                    nc.sync.dma_start(out=be_sb[:], in_=_ap(beta, 0, [[0, P], [1, D]]))

                ys = []
                for rt in range(4):
                    y = ty.tile([P, 2, 512], F32, tag="y", name=f"y{rt}")
                    ys.append(y)
                pos = {}
                for dc in range(2):
                    for rt in range(4):
                        po = tps.tile([P, 512], F32, tag=f"po{dc}_{rt}",
                                      name=f"po{dc}_{rt}")
                        pos[(dc, rt)] = po
                wots = {}
                for jt in range(NDT):
                    for dc in range(2):
                        wot = two.tile([P, 512], F32R, tag="wo",
                                       name=f"wo{jt}_{dc}")
                        nc.sync.dma_start(
                            out=wot[:],
                            in_=woT[jt * P:(jt + 1) * P, dc * 512:(dc + 1) * 512])
                        wots[dc] = wot
                    for rt in range(4):
                        for dc in range(2):
                            nc.tensor.matmul(
                                pos[(dc, rt)][:], ctx_sb[:, jt, rt * P:(rt + 1) * P],
                                wots[dc][:],
                                start=(jt == 0),
                                stop=(jt == NDT - 1 and not has_out_bias))
                for dc in range(2):
                    for rt in range(4):
                        if has_out_bias:
                            nc.tensor.matmul(
                                pos[(dc, rt)][:], ones_col[:],
                                b_sb[:, 3 * D + dc * 512:3 * D + (dc + 1) * 512],
                                start=False, stop=True)
                        nc.vector.tensor_add(
                            ys[rt][:, dc, :], pos[(dc, rt)][:],
                            xr_sb[:, rt, dc * 512:(dc + 1) * 512])
                for rt in range(4):
                    y = ys[rt]
                    stats = tt.tile([P, 2, nc.vector.BN_STATS_DIM], F32, tag="st")
                    mv = tt.tile([P, nc.vector.BN_AGGR_DIM], F32, tag="mv")
                    for sg in range(2):
                        nc.vector.bn_stats(out=stats[:, sg, :], in_=y[:, sg, :])
                    nc.vector.bn_aggr(out=mv[:], in_=stats[:])
                    sd = tt.tile([P, 1], F32, tag="sd")
                    rstd = tt.tile([P, 1], F32, tag="rs")
                    nc.scalar.activation(sd[:], mv[:, 1:2], AF.Sqrt, bias=eps_sb[:])
                    nc.vector.reciprocal(rstd[:], sd[:])
                    yn = tt.tile([P, D], F32, tag="yn")
                    nc.vector.tensor_scalar(
                        out=yn[:], in0=_ap(y, 0, [[2 * 512, P], [1, D]]),
                        scalar1=mv[:, 0:1], scalar2=rstd[:],
                        op0=mybir.AluOpType.subtract, op1=mybir.AluOpType.mult)
                    if has_gamma:
                        nc.vector.tensor_mul(yn[:], yn[:], g_sb[:])
                    if has_beta:
                        nc.vector.tensor_add(yn[:], yn[:], be_sb[:])
                    nc.sync.dma_start(out=out_c[rt * P:(rt + 1) * P, :], in_=yn[:])

    nc.compile()
    return nc


_PROGRAM_CACHE = {}


def _get_program(flags):
    if flags not in _PROGRAM_CACHE:
        _PROGRAM_CACHE[flags] = build_program(*flags)
    return _PROGRAM_CACHE[flags]


def make_in_maps(x, in_proj_w, in_proj_b, out_proj_w, out_proj_b, ln_gamma, ln_beta):
    x = np.asarray(x, np.float32)
    in_proj_w = np.asarray(in_proj_w, np.float32)
    in_proj_b = np.asarray(in_proj_b, np.float32)
    out_proj_w = np.asarray(out_proj_w, np.float32)
    out_proj_b = np.asarray(out_proj_b, np.float32)
    ln_gamma = np.asarray(ln_gamma, np.float32)
    ln_beta = np.asarray(ln_beta, np.float32)

    scale = 1.0 / np.sqrt(HD)
    wqT = np.ascontiguousarray(in_proj_w[:D].T) * scale
    wkT = np.ascontiguousarray(in_proj_w[D:2 * D].T)
    wvT = np.ascontiguousarray(in_proj_w[2 * D:].T)
    woT = np.ascontiguousarray(out_proj_w.T)
    bvec = np.concatenate(
        [in_proj_b[:D] * scale, in_proj_b[D:], out_proj_b]).astype(np.float32)

    shared = dict(wqT=wqT, wkT=wkT, wvT=wvT, woT=woT, bvec=bvec,
                  gamma=ln_gamma, beta=ln_beta)
    in_maps = []
    for c in range(NCORES):
        b, sh = c // 4, c % 4
        xb = x[b]
        xbT = np.ascontiguousarray(xb.T)
        m = dict(shared)
        m["xT"] = xbT
        m["xQT"] = np.ascontiguousarray(xbT[:, sh * R:(sh + 1) * R])
        m["xrow"] = np.ascontiguousarray(xb[sh * R:(sh + 1) * R])
        in_maps.append(m)

    flags = (bool(in_proj_b.any()), bool(out_proj_b.any()),
             bool((ln_gamma != 1.0).any()), bool(ln_beta.any()))
    return in_maps, flags


def assemble(results):
    out = np.empty((B, S, D), np.float32)
    attn = np.empty((B, S, S), np.float32)
    for c in range(NCORES):
        b, sh = c // 4, c % 4
        out[b, sh * R:(sh + 1) * R] = results[c]["out_c"]
        attn[b, sh * R:(sh + 1) * R] = \
            results[c]["wmeanT"].astype(np.float32).T / H
    return out, attn


def kernel(x, in_proj_w, in_proj_b, out_proj_w, out_proj_b, ln_gamma, ln_beta):
    in_maps, flags = make_in_maps(
        x, in_proj_w, in_proj_b, out_proj_w, out_proj_b, ln_gamma, ln_beta)
    nc = _get_program(flags)
    res = run_bass_kernel_spmd(nc, in_maps, list(range(NCORES)))
    return assemble(res.results)


# revision 27
# speedup vs baseline: 1.0164x; 1.0164x over previous
"""Trainium2 Bass kernel: MultiHeadSelfAttention + residual + LayerNorm.

Problem: x[2,2048,1024], H=16 heads, in_proj -> attention -> out_proj ->
residual -> LayerNorm; also returns mean-over-heads attention weights.

Sharding (8 cores, no collectives):
  core c handles batch b=c//4 and query rows [sh*512,(sh+1)*512), sh=c%4.
  K/V projections are computed redundantly per batch group (4 cores share a
  batch, each computes the full K/V for its batch: +8.6 GFLOP/core, which is
  cheaper than any cross-core exchange at trn2 collective rates).

Layouts (all chosen so every matmul contracts over the partition dim):
  qkv^T:  Q^T[dq, rows], K^T[dk, rows] via W^T-stationary matmuls
  V:      [rows, dv]     via x^T-stationary matmuls
  scores^T[k, q] = (K^T tile).T @ Q^T  (head pairs packed in PE row groups)
  exp on ACT (PSUM->SBUF, fp16), denominator fused into the ctx matmul by
  augmenting V with a ones-column block: out rows = [ctx^T | dup(denom)].
  ctx^T scaled by 1/denom, out_proj with ctx^T-stationary -> straight [row, d]
  layout so residual+LayerNorm use bn_stats along the free dim.
  attn-weight mean accumulated as sum_h e_h * (1/denom_h) in fp16; the /H and
  the final [k,q]->[q,k] transpose happen on the host.
"""

import sys

for _p in ("/opt/trn_rl_repo",):
    if _p not in sys.path:
        sys.path.insert(0, _p)

import numpy as np

import concourse.bass as bass
from concourse import bacc
import concourse.mybir as mybir
import concourse.tile as tile
from concourse.bass_utils import run_bass_kernel_spmd
F32 = mybir.dt.float32
F32R = mybir.dt.float32r
F16 = mybir.dt.float16
AF = mybir.ActivationFunctionType

NCORES = 8
B, S, D = 2, 2048, 1024
H, HD = 16, 64
P = 128
R = S // 4            # 512 query rows per core
NKT = S // P          # 16 k tiles
NDT = D // P          # 8 d tiles
NHP = H // 2          # 8 head pairs
LN_EPS = 1e-5

# V sbuf free-dim layout: per k-tile, 16 heads of [V_h(64) | ones(1)] = 1040
VHW = 65                    # per-head width (64 V cols + 1 ones col)
VKT = H * VHW               # 1040 cols per k-tile group
VCOLS = NKT * VKT


def _ap(t, extra_offset, ap_dims):
    return bass.AP(tensor=t.tensor, offset=t.offset + extra_offset, ap=ap_dims)


def build_program(has_qkv_bias, has_out_bias, has_gamma, has_beta):
    nc = bacc.Bacc("TRN2", target_bir_lowering=False, debug=False)

    xT = nc.dram_tensor("xT", [D, S], F32R, kind="ExternalInput").ap()
    xQT = nc.dram_tensor("xQT", [D, R], F32R, kind="ExternalInput").ap()
    xrow = nc.dram_tensor("xrow", [R, D], F32, kind="ExternalInput").ap()
    wqT = nc.dram_tensor("wqT", [D, D], F32R, kind="ExternalInput").ap()
    wkT = nc.dram_tensor("wkT", [D, D], F32R, kind="ExternalInput").ap()
    wvT = nc.dram_tensor("wvT", [D, D], F32R, kind="ExternalInput").ap()
    woT = nc.dram_tensor("woT", [D, D], F32R, kind="ExternalInput").ap()
    # bias layout: [bq/8, bk, bv, bo] (4*D)
    bvec = nc.dram_tensor("bvec", [4 * D], F32R, kind="ExternalInput").ap()
    gamma = nc.dram_tensor("gamma", [D], F32, kind="ExternalInput").ap()
    beta = nc.dram_tensor("beta", [D], F32, kind="ExternalInput").ap()

    out_c = nc.dram_tensor("out_c", [R, D], F32, kind="ExternalOutput").ap()
    wmeanT = nc.dram_tensor("wmeanT", [S, R], F16, kind="ExternalOutput").ap()

    kT_dram = nc.dram_tensor("kT_scratch", [NDT, P, S], F32R)

    with tile.TileContext(nc) as tc:
        with tc.tile_pool(name="persist", bufs=1) as persist:
            v_sb = persist.tile([P, VCOLS], F16)
            q_sb = persist.tile([P, NDT, R], F32R)
            ctx_sb = persist.tile([P, NHP, R], F32R)
            wacc = persist.tile([P, NKT, R], F16)
            wacc2 = persist.tile([P, NKT, R], F16)
            nc.vector.memset(
                _ap(v_sb, 64, [[VCOLS, P], [VHW, NKT * H], [1, 1]]), 1.0)
            b_sb = ones_col = ones_row = None
            if has_qkv_bias or has_out_bias:
                b_sb = persist.tile([1, 4 * D], F32R)
                ones_colf = persist.tile([1, P], F32)
                ones_rowf = persist.tile([1, 512], F32)
                ones_col = persist.tile([1, P], F32R)
                ones_row = persist.tile([1, 512], F32R)
                nc.sync.dma_start(out=b_sb[:], in_=_ap(bvec, 0, [[0, 1], [1, 4 * D]]))
                nc.vector.memset(ones_colf[:], 1.0)
                nc.vector.memset(ones_rowf[:], 1.0)
                nc.vector.tensor_copy(ones_col[:], ones_colf[:])
                nc.vector.tensor_copy(ones_row[:], ones_rowf[:])

            # ---------------- Phase 1: projections ----------------
            # Q first (smallest inputs -> earliest PE start)
            with tc.tile_pool(name="projq_sbuf", bufs=1) as pq, \
                 tc.tile_pool(name="projq_w", bufs=12) as qwpool, \
                 tc.tile_pool(name="projq_ps", bufs=4, space="PSUM") as qps:
                xQT_sb = pq.tile([P, NDT, R], F32R)
                for dt in range(NDT):
                    nc.sync.dma_start(out=xQT_sb[:, dt, :], in_=xQT[dt * P:(dt + 1) * P, :])
                for ft in range(NDT):
                    qw = []
                    for dt in range(NDT):
                        qwt = qwpool.tile([P, P], F32R, tag="kw")
                        nc.sync.dma_start(
                            out=qwt[:],
                            in_=wqT[dt * P:(dt + 1) * P, ft * P:(ft + 1) * P])
                        qw.append(qwt)
                    ps = qps.tile([P, 512], F32, tag="ps")
                    for dt in range(NDT):
                        nc.tensor.matmul(
                            ps[:], qw[dt][:], xQT_sb[:, dt, :],
                            start=(dt == 0),
                            stop=(dt == NDT - 1 and not has_qkv_bias))
                    if has_qkv_bias:
                        nc.tensor.matmul(
                            ps[:], b_sb[:, ft * P:(ft + 1) * P], ones_row[:],
                            start=False, stop=True)
                    nc.vector.tensor_copy(q_sb[:, ft, :], ps[:])

            # K^T and V projections, x^T streamed in sequence-quarters
            with tc.tile_pool(name="proj_sbuf", bufs=1) as pj, \
                 tc.tile_pool(name="projx", bufs=2) as pxpool, \
                 tc.tile_pool(name="wtiles", bufs=3) as wpool, \
                 tc.tile_pool(name="proj_ps", bufs=4, space="PSUM") as pps:
                kw_sb = pj.tile([P, NDT, D], F32R)
                wv_sb = pj.tile([P, NDT, D], F32R)
                for dt in range(NDT):
                    nc.sync.dma_start(out=kw_sb[:, dt, :], in_=wkT[dt * P:(dt + 1) * P, :])
                    nc.sync.dma_start(out=wv_sb[:, dt, :], in_=wvT[dt * P:(dt + 1) * P, :])
                for qp in range(2):
                    xhs = []
                    for qq in range(2):
                        quarter = qp * 2 + qq
                        xh = pxpool.tile([P, NDT, 512], F32R, tag="xTh",
                                         name=f"xh{quarter}")
                        for dt in range(NDT):
                            nc.sync.dma_start(
                                out=xh[:, dt, :],
                                in_=xT[dt * P:(dt + 1) * P,
                                       quarter * 512:quarter * 512 + 512])
                        xhs.append(xh)

                    # K^T -> DRAM scratch; stationary kw reused across the
                    # two resident quarters
                    for ft in range(NDT):
                        pss = [pps.tile([P, 512], F32, tag="ps",
                                        name=f"kps{ft}_{qq}") for qq in range(2)]
                        for dt in range(NDT):
                            for qq in range(2):
                                nc.tensor.matmul(
                                    pss[qq][:], kw_sb[:, dt, ft * P:(ft + 1) * P],
                                    xhs[qq][:, dt, :],
                                    start=(dt == 0),
                                    stop=(dt == NDT - 1 and not has_qkv_bias))
                        for qq in range(2):
                            if has_qkv_bias:
                                nc.tensor.matmul(
                                    pss[qq][:], b_sb[:, D + ft * P:D + (ft + 1) * P],
                                    ones_row[:], start=False, stop=True)
                            kst = wpool.tile([P, 512], F32R, tag="kst")
                            nc.vector.tensor_copy(kst[:], pss[qq][:])
                            h0 = (qp * 2 + qq) * 512
                            nc.gpsimd.dma_start(
                                out=kT_dram[ft, :, h0:h0 + 512], in_=kst[:])

                    # V -> SBUF fp16; stationary xh tile reused across fc
                    for qq in range(2):
                        quarter = qp * 2 + qq
                        for rtq in range(4):
                            rt = quarter * 4 + rtq
                            pss = [pps.tile([P, 512], F32, tag="ps",
                                            name=f"vps{rt}_{fc}") for fc in range(2)]
                            for dt in range(NDT):
                                for fc in range(2):
                                    nc.tensor.matmul(
                                        pss[fc][:], xhs[qq][:, dt, rtq * P:(rtq + 1) * P],
                                        wv_sb[:, dt, fc * 512:(fc + 1) * 512],
                                        start=(dt == 0),
                                        stop=(dt == NDT - 1 and not has_qkv_bias))
                            for fc in range(2):
                                if has_qkv_bias:
                                    nc.tensor.matmul(
                                        pss[fc][:], ones_col[:],
                                        b_sb[:, 2 * D + fc * 512:2 * D + (fc + 1) * 512],
                                        start=False, stop=True)
                                nc.vector.tensor_copy(
                                    _ap(v_sb, rt * VKT + fc * 8 * VHW,
                                        [[VCOLS, P], [VHW, 8], [1, 64]]),
                                    pss[fc][:])

            # ---------------- Phase 2: attention ----------------
            chunks = [(0, 3), (3, 3), (6, 3), (9, 3), (12, 3), (15, 1)]
            with tc.tile_pool(name="attn_sbuf", bufs=4) as ap_, \
                 tc.tile_pool(name="kp_pool", bufs=2) as kpool, \
                 tc.tile_pool(name="r_pool", bufs=2) as rpool, \
                 tc.tile_pool(name="wt_pool", bufs=2) as wtpool, \
                 tc.tile_pool(name="score_ps", bufs=2, space="PSUM") as sps, \
                 tc.tile_pool(name="ctx_ps", bufs=2, space="PSUM") as cps:
                for hp in range(NHP):
                    kp = kpool.tile([P, S], F32R, tag="kp")
                    nc.sync.dma_start(out=kp[:], in_=kT_dram[hp])
                    ctx_ps = {}
                    e_sb = {}
                    for h in (2 * hp, 2 * hp + 1):
                        ctx_ps[h] = cps.tile([65, R], F32, tag="ctx", name=f"ctx{h}")
                        e_sb[h] = ap_.tile([P, NKT, R], F16, tag="e", name=f"e{h}")
                    for (k0, klen) in chunks:
                        for h in (2 * hp, 2 * hp + 1):
                            hb = (h % 2) * 64
                            ps = sps.tile([P, 3, R], F32, tag="sc")
                            for i in range(klen):
                                kt = k0 + i
                                nc.tensor.matmul(
                                    ps[:, i, :],
                                    kp[hb:hb + 64, kt * P:(kt + 1) * P],
                                    q_sb[hb:hb + 64, hp, :],
                                    start=True, stop=True)
                            nc.scalar.activation(
                                e_sb[h][:, k0:k0 + klen, :], ps[:, 0:klen, :], AF.Exp)
                            # ctx accumulation: lhsT = [V_h(64) | ones(1)]
                            # contiguous -> psum rows 0:64 ctx, row 64 denom
                            for i in range(klen):
                                kt = k0 + i
                                vstart = kt * VKT + h * VHW
                                nc.tensor.matmul(
                                    ctx_ps[h][:], v_sb[:, vstart:vstart + VHW],
                                    e_sb[h][:, kt, :],
                                    start=(kt == 0), stop=(kt == NKT - 1))
                    for h in (2 * hp, 2 * hp + 1):
                        # ctx rows 0:64, denominator in row 64
                        r1 = rpool.tile([P, R], F16, tag="r1")
                        r128 = rpool.tile([P, R], F16, tag="r128")
                        with nc.allow_low_precision(reason="softmax recip to fp16"):
                            nc.vector.reciprocal(r1[64:65, :], ctx_ps[h][64:65, :])
                        nc.sync.dma_start(out=r1[0:1, :], in_=r1[64:65, :])
                        nc.gpsimd.partition_broadcast(r128[:], r1[0:1, :])
                        if h % 2 == 0:
                            nc.vector.tensor_mul(
                                ctx_sb[0:64, hp, :], ctx_ps[h][0:64, :], r128[0:64, :])
                        else:
                            ctmp = rpool.tile([P, R], F32R, tag="ctmp")
                            nc.vector.tensor_mul(
                                ctmp[0:64, :], ctx_ps[h][0:64, :], r128[0:64, :])
                            nc.sync.dma_start(
                                out=ctx_sb[64:128, hp, :], in_=ctmp[0:64, :])
                        # attn-weight mean accumulation (fp16):
                        #   wacc += e_h * r128   (first head: plain write)
                        acc = wacc if h % 2 == 0 else wacc2
                        for half in range(4):
                            sl = slice(half * 4, half * 4 + 4)
                            rb = _ap(r128, 0, [[R, P], [0, 4], [1, R]])
                            if h < 2:
                                nc.vector.tensor_mul(
                                    acc[:, sl, :], e_sb[h][:, sl, :], rb)
                            else:
                                wt = wtpool.tile([P, 4, R], F16, tag="wt")
                                nc.vector.tensor_mul(wt[:], e_sb[h][:, sl, :], rb)
                                nc.vector.tensor_add(
                                    acc[:, sl, :], acc[:, sl, :], wt[:])

            for half in range(2):
                sl = slice(half * 8, half * 8 + 8)
                nc.vector.tensor_add(wacc[:, sl, :], wacc[:, sl, :],
                                     wacc2[:, sl, :])
            nc.sync.dma_start(
                out=wmeanT.rearrange("(kt p) q -> p kt q", p=P), in_=wacc[:])

            # ---------------- Phase 3: out_proj + residual + LayerNorm ------
            with tc.tile_pool(name="tail_sbuf", bufs=1) as tl, \
                 tc.tile_pool(name="tail_y", bufs=4) as ty, \
                 tc.tile_pool(name="tail_wo", bufs=4) as two, \
                 tc.tile_pool(name="tail_tmp", bufs=3) as tt, \
                 tc.tile_pool(name="tail_ps", bufs=1, space="PSUM") as tps:
                xr_sb = tl.tile([P, 4, D], F32)
                eps_sb = tl.tile([P, 1], F32)
                nc.vector.memset(eps_sb[:], LN_EPS)
                for rt in range(4):
                    nc.sync.dma_start(
                        out=xr_sb[:, rt, :], in_=xrow[rt * P:(rt + 1) * P, :])
                if has_gamma:
                    g_sb = tl.tile([P, D], F32)
                    nc.sync.dma_start(out=g_sb[:], in_=_ap(gamma, 0, [[0, P], [1, D]]))
                if has_beta:
                    be_sb = tl.tile([P, D], F32)
                    nc.sync.dma_start(out=be_sb[:], in_=_ap(beta, 0, [[0, P], [1, D]]))

                wo_sb = tl.tile([P, NDT, D], F32R)
                for jt in range(NDT):
                    nc.sync.dma_start(
                        out=wo_sb[:, jt, :], in_=woT[jt * P:(jt + 1) * P, :])
                ys = []
                for rt in range(4):
                    y = ty.tile([P, 2, 512], F32, tag="y", name=f"y{rt}")
                    ys.append(y)
                    pos = []
                    for dc in range(2):
                        po = tps.tile([P, 512], F32, tag="po", name=f"po{rt}_{dc}",
                                      bufs=4)
                        pos.append(po)
                    for jt in range(NDT):
                        for dc in range(2):
                            nc.tensor.matmul(
                                pos[dc][:], ctx_sb[:, jt, rt * P:(rt + 1) * P],
                                wo_sb[:, jt, dc * 512:(dc + 1) * 512],
                                start=(jt == 0),
                                stop=(jt == NDT - 1 and not has_out_bias))
                    for dc in range(2):
                        if has_out_bias:
                            nc.tensor.matmul(
                                pos[dc][:], ones_col[:],
                                b_sb[:, 3 * D + dc * 512:3 * D + (dc + 1) * 512],
                                start=False, stop=True)
                        nc.vector.tensor_add(
                            y[:, dc, :], pos[dc][:],
                            xr_sb[:, rt, dc * 512:(dc + 1) * 512])
                for rt in range(4):
                    y = ys[rt]
                    stats = tt.tile([P, 2, nc.vector.BN_STATS_DIM], F32, tag="st")
                    mv = tt.tile([P, nc.vector.BN_AGGR_DIM], F32, tag="mv")
                    for sg in range(2):
                        nc.vector.bn_stats(out=stats[:, sg, :], in_=y[:, sg, :])
                    nc.vector.bn_aggr(out=mv[:], in_=stats[:])
                    sd = tt.tile([P, 1], F32, tag="sd")
                    rstd = tt.tile([P, 1], F32, tag="rs")
                    nc.scalar.activation(sd[:], mv[:, 1:2], AF.Sqrt, bias=eps_sb[:])
                    nc.vector.reciprocal(rstd[:], sd[:])
                    yn = tt.tile([P, D], F32, tag="yn")
                    nc.vector.tensor_scalar(
                        out=yn[:], in0=_ap(y, 0, [[2 * 512, P], [1, D]]),
                        scalar1=mv[:, 0:1], scalar2=rstd[:],
                        op0=mybir.AluOpType.subtract, op1=mybir.AluOpType.mult)
                    if has_gamma:
                        nc.vector.tensor_mul(yn[:], yn[:], g_sb[:])
                    if has_beta:
                        nc.vector.tensor_add(yn[:], yn[:], be_sb[:])
                    nc.sync.dma_start(out=out_c[rt * P:(rt + 1) * P, :], in_=yn[:])

    nc.compile()
    return nc


_PROGRAM_CACHE = {}


def _get_program(flags):
    if flags not in _PROGRAM_CACHE:
        _PROGRAM_CACHE[flags] = build_program(*flags)
    return _PROGRAM_CACHE[flags]


def make_in_maps(x, in_proj_w, in_proj_b, out_proj_w, out_proj_b, ln_gamma, ln_beta):
    x = np.asarray(x, np.float32)
    in_proj_w = np.asarray(in_proj_w, np.float32)
    in_proj_b = np.asarray(in_proj_b, np.float32)
    out_proj_w = np.asarray(out_proj_w, np.float32)
    out_proj_b = np.asarray(out_proj_b, np.float32)
    ln_gamma = np.asarray(ln_gamma, np.float32)
    ln_beta = np.asarray(ln_beta, np.float32)

    scale = 1.0 / np.sqrt(HD)
    wqT = np.ascontiguousarray(in_proj_w[:D].T) * scale
    wkT = np.ascontiguousarray(in_proj_w[D:2 * D].T)
    wvT = np.ascontiguousarray(in_proj_w[2 * D:].T)
    woT = np.ascontiguousarray(out_proj_w.T)
    bvec = np.concatenate(
        [in_proj_b[:D] * scale, in_proj_b[D:], out_proj_b]).astype(np.float32)

    shared = dict(wqT=wqT, wkT=wkT, wvT=wvT, woT=woT, bvec=bvec,
                  gamma=ln_gamma, beta=ln_beta)
    in_maps = []
    for c in range(NCORES):
        b, sh = c // 4, c % 4
        xb = x[b]
        xbT = np.ascontiguousarray(xb.T)
        m = dict(shared)
        m["xT"] = xbT
        m["xQT"] = np.ascontiguousarray(xbT[:, sh * R:(sh + 1) * R])
        m["xrow"] = np.ascontiguousarray(xb[sh * R:(sh + 1) * R])
        in_maps.append(m)

    flags = (bool(in_proj_b.any()), bool(out_proj_b.any()),
             bool((ln_gamma != 1.0).any()), bool(ln_beta.any()))
    return in_maps, flags


def assemble(results):
    out = np.empty((B, S, D), np.float32)
    attn = np.empty((B, S, S), np.float32)
    for c in range(NCORES):
        b, sh = c // 4, c % 4
        out[b, sh * R:(sh + 1) * R] = results[c]["out_c"]
        attn[b, sh * R:(sh + 1) * R] = \
            results[c]["wmeanT"].astype(np.float32).T / H
    return out, attn


def kernel(x, in_proj_w, in_proj_b, out_proj_w, out_proj_b, ln_gamma, ln_beta):
    in_maps, flags = make_in_maps(
        x, in_proj_w, in_proj_b, out_proj_w, out_proj_b, ln_gamma, ln_beta)
    nc = _get_program(flags)
    res = run_bass_kernel_spmd(nc, in_maps, list(range(NCORES)))
    return assemble(res.results)


# revision 28
# speedup vs baseline: 1.0206x; 1.0041x over previous
"""Trainium2 Bass kernel: MultiHeadSelfAttention + residual + LayerNorm.

Problem: x[2,2048,1024], H=16 heads, in_proj -> attention -> out_proj ->
residual -> LayerNorm; also returns mean-over-heads attention weights.

Sharding (8 cores, no collectives):
  core c handles batch b=c//4 and query rows [sh*512,(sh+1)*512), sh=c%4.
  K/V projections are computed redundantly per batch group (4 cores share a
  batch, each computes the full K/V for its batch: +8.6 GFLOP/core, which is
  cheaper than any cross-core exchange at trn2 collective rates).

Layouts (all chosen so every matmul contracts over the partition dim):
  qkv^T:  Q^T[dq, rows], K^T[dk, rows] via W^T-stationary matmuls
  V:      [rows, dv]     via x^T-stationary matmuls
  scores^T[k, q] = (K^T tile).T @ Q^T  (head pairs packed in PE row groups)
  exp on ACT (PSUM->SBUF, fp16), denominator fused into the ctx matmul by
  augmenting V with a ones-column block: out rows = [ctx^T | dup(denom)].
  ctx^T scaled by 1/denom, out_proj with ctx^T-stationary -> straight [row, d]
  layout so residual+LayerNorm use bn_stats along the free dim.
  attn-weight mean accumulated as sum_h e_h * (1/denom_h) in fp16; the /H and
  the final [k,q]->[q,k] transpose happen on the host.
"""

import sys

for _p in ("/opt/trn_rl_repo",):
    if _p not in sys.path:
        sys.path.insert(0, _p)

import numpy as np

import concourse.bass as bass
from concourse import bacc
import concourse.mybir as mybir
import concourse.tile as tile
from concourse.bass_utils import run_bass_kernel_spmd
F32 = mybir.dt.float32
F32R = mybir.dt.float32r
F16 = mybir.dt.float16
AF = mybir.ActivationFunctionType

NCORES = 8
B, S, D = 2, 2048, 1024
H, HD = 16, 64
P = 128
R = S // 4            # 512 query rows per core
NKT = S // P          # 16 k tiles
NDT = D // P          # 8 d tiles
NHP = H // 2          # 8 head pairs
LN_EPS = 1e-5

# V sbuf free-dim layout: per k-tile, 16 heads of [V_h(64) | ones(1)] = 1040
VHW = 65                    # per-head width (64 V cols + 1 ones col)
VKT = H * VHW               # 1040 cols per k-tile group
VCOLS = NKT * VKT


def _ap(t, extra_offset, ap_dims):
    return bass.AP(tensor=t.tensor, offset=t.offset + extra_offset, ap=ap_dims)


def build_program(has_qkv_bias, has_out_bias, has_gamma, has_beta):
    nc = bacc.Bacc("TRN2", target_bir_lowering=False, debug=False)

    xT = nc.dram_tensor("xT", [D, S], F32R, kind="ExternalInput").ap()
    xQT = nc.dram_tensor("xQT", [D, R], F32R, kind="ExternalInput").ap()
    xrow = nc.dram_tensor("xrow", [R, D], F32, kind="ExternalInput").ap()
    wqT = nc.dram_tensor("wqT", [D, D], F32R, kind="ExternalInput").ap()
    wkT = nc.dram_tensor("wkT", [D, D], F32R, kind="ExternalInput").ap()
    wvT = nc.dram_tensor("wvT", [D, D], F32R, kind="ExternalInput").ap()
    woT = nc.dram_tensor("woT", [D, D], F32R, kind="ExternalInput").ap()
    # bias layout: [bq/8, bk, bv, bo] (4*D)
    bvec = nc.dram_tensor("bvec", [4 * D], F32R, kind="ExternalInput").ap()
    gamma = nc.dram_tensor("gamma", [D], F32, kind="ExternalInput").ap()
    beta = nc.dram_tensor("beta", [D], F32, kind="ExternalInput").ap()

    out_c = nc.dram_tensor("out_c", [R, D], F32, kind="ExternalOutput").ap()
    wmeanT = nc.dram_tensor("wmeanT", [S, R], F16, kind="ExternalOutput").ap()

    kT_dram = nc.dram_tensor("kT_scratch", [NDT, P, S], F32R)

    with tile.TileContext(nc) as tc:
        with tc.tile_pool(name="persist", bufs=1) as persist:
            v_sb = persist.tile([P, VCOLS], F16)
            q_sb = persist.tile([P, NDT, R], F32R)
            ctx_sb = persist.tile([P, NHP, R], F32R)
            wacc = persist.tile([P, NKT, R], F16)
            wacc2 = persist.tile([P, NKT, R], F16)
            nc.vector.memset(
                _ap(v_sb, 64, [[VCOLS, P], [VHW, NKT * H], [1, 1]]), 1.0)
            b_sb = ones_col = ones_row = None
            if has_qkv_bias or has_out_bias:
                b_sb = persist.tile([1, 4 * D], F32R)
                ones_colf = persist.tile([1, P], F32)
                ones_rowf = persist.tile([1, 512], F32)
                ones_col = persist.tile([1, P], F32R)
                ones_row = persist.tile([1, 512], F32R)
                nc.sync.dma_start(out=b_sb[:], in_=_ap(bvec, 0, [[0, 1], [1, 4 * D]]))
                nc.vector.memset(ones_colf[:], 1.0)
                nc.vector.memset(ones_rowf[:], 1.0)
                nc.vector.tensor_copy(ones_col[:], ones_colf[:])
                nc.vector.tensor_copy(ones_row[:], ones_rowf[:])

            # ---------------- Phase 1: projections ----------------
            # Q first (smallest inputs -> earliest PE start)
            with tc.tile_pool(name="projq_sbuf", bufs=1) as pq, \
                 tc.tile_pool(name="projq_w", bufs=12) as qwpool, \
                 tc.tile_pool(name="projq_ps", bufs=4, space="PSUM") as qps:
                xQT_sb = pq.tile([P, NDT, R], F32R)
                for dt in range(NDT):
                    nc.sync.dma_start(out=xQT_sb[:, dt, :], in_=xQT[dt * P:(dt + 1) * P, :])
                for ft in range(NDT):
                    qw = []
                    for dt in range(NDT):
                        qwt = qwpool.tile([P, P], F32R, tag="kw")
                        nc.sync.dma_start(
                            out=qwt[:],
                            in_=wqT[dt * P:(dt + 1) * P, ft * P:(ft + 1) * P])
                        qw.append(qwt)
                    ps = qps.tile([P, 512], F32, tag="ps")
                    for dt in range(NDT):
                        nc.tensor.matmul(
                            ps[:], qw[dt][:], xQT_sb[:, dt, :],
                            start=(dt == 0),
                            stop=(dt == NDT - 1 and not has_qkv_bias))
                    if has_qkv_bias:
                        nc.tensor.matmul(
                            ps[:], b_sb[:, ft * P:(ft + 1) * P], ones_row[:],
                            start=False, stop=True)
                    nc.vector.tensor_copy(q_sb[:, ft, :], ps[:])

            # K^T and V projections, x^T streamed in sequence-quarters
            with tc.tile_pool(name="proj_sbuf", bufs=1) as pj, \
                 tc.tile_pool(name="projx", bufs=2) as pxpool, \
                 tc.tile_pool(name="wtiles", bufs=3) as wpool, \
                 tc.tile_pool(name="proj_ps", bufs=4, space="PSUM") as pps:
                kw_sb = pj.tile([P, NDT, D], F32R)
                wv_sb = pj.tile([P, NDT, D], F32R)
                for dt in range(NDT):
                    nc.sync.dma_start(out=kw_sb[:, dt, :], in_=wkT[dt * P:(dt + 1) * P, :])
                for dt in range(NDT):
                    nc.sync.dma_start(out=wv_sb[:, dt, :], in_=wvT[dt * P:(dt + 1) * P, :])
                for qp in range(2):
                    xhs = []
                    for qq in range(2):
                        quarter = qp * 2 + qq
                        xh = pxpool.tile([P, NDT, 512], F32R, tag="xTh",
                                         name=f"xh{quarter}")
                        for dt in range(NDT):
                            nc.sync.dma_start(
                                out=xh[:, dt, :],
                                in_=xT[dt * P:(dt + 1) * P,
                                       quarter * 512:quarter * 512 + 512])
                        xhs.append(xh)

                    # K^T -> DRAM scratch; stationary kw reused across the
                    # two resident quarters
                    for ft in range(NDT):
                        pss = [pps.tile([P, 512], F32, tag="ps",
                                        name=f"kps{ft}_{qq}") for qq in range(2)]
                        for dt in range(NDT):
                            for qq in range(2):
                                nc.tensor.matmul(
                                    pss[qq][:], kw_sb[:, dt, ft * P:(ft + 1) * P],
                                    xhs[qq][:, dt, :],
                                    start=(dt == 0),
                                    stop=(dt == NDT - 1 and not has_qkv_bias))
                        for qq in range(2):
                            if has_qkv_bias:
                                nc.tensor.matmul(
                                    pss[qq][:], b_sb[:, D + ft * P:D + (ft + 1) * P],
                                    ones_row[:], start=False, stop=True)
                            kst = wpool.tile([P, 512], F32R, tag="kst")
                            nc.vector.tensor_copy(kst[:], pss[qq][:])
                            h0 = (qp * 2 + qq) * 512
                            nc.gpsimd.dma_start(
                                out=kT_dram[ft, :, h0:h0 + 512], in_=kst[:])

                    # V -> SBUF fp16; stationary xh tile reused across fc
                    for qq in range(2):
                        quarter = qp * 2 + qq
                        for rtq in range(4):
                            rt = quarter * 4 + rtq
                            pss = [pps.tile([P, 512], F32, tag="ps",
                                            name=f"vps{rt}_{fc}") for fc in range(2)]
                            for dt in range(NDT):
                                for fc in range(2):
                                    nc.tensor.matmul(
                                        pss[fc][:], xhs[qq][:, dt, rtq * P:(rtq + 1) * P],
                                        wv_sb[:, dt, fc * 512:(fc + 1) * 512],
                                        start=(dt == 0),
                                        stop=(dt == NDT - 1 and not has_qkv_bias))
                            for fc in range(2):
                                if has_qkv_bias:
                                    nc.tensor.matmul(
                                        pss[fc][:], ones_col[:],
                                        b_sb[:, 2 * D + fc * 512:2 * D + (fc + 1) * 512],
                                        start=False, stop=True)
                                nc.vector.tensor_copy(
                                    _ap(v_sb, rt * VKT + fc * 8 * VHW,
                                        [[VCOLS, P], [VHW, 8], [1, 64]]),
                                    pss[fc][:])

            # ---------------- Phase 2: attention ----------------
            chunks = [(0, 3), (3, 3), (6, 3), (9, 3), (12, 3), (15, 1)]
            with tc.tile_pool(name="attn_sbuf", bufs=4) as ap_, \
                 tc.tile_pool(name="kp_pool", bufs=2) as kpool, \
                 tc.tile_pool(name="r_pool", bufs=2) as rpool, \
                 tc.tile_pool(name="wt_pool", bufs=2) as wtpool, \
                 tc.tile_pool(name="score_ps", bufs=2, space="PSUM") as sps, \
                 tc.tile_pool(name="ctx_ps", bufs=2, space="PSUM") as cps:
                for hp in range(NHP):
                    kp = kpool.tile([P, S], F32R, tag="kp")
                    nc.sync.dma_start(out=kp[:], in_=kT_dram[hp])
                    ctx_ps = {}
                    e_sb = {}
                    for h in (2 * hp, 2 * hp + 1):
                        ctx_ps[h] = cps.tile([65, R], F32, tag="ctx", name=f"ctx{h}")
                        e_sb[h] = ap_.tile([P, NKT, R], F16, tag="e", name=f"e{h}")
                    for (k0, klen) in chunks:
                        for h in (2 * hp, 2 * hp + 1):
                            hb = (h % 2) * 64
                            ps = sps.tile([P, 3, R], F32, tag="sc")
                            for i in range(klen):
                                kt = k0 + i
                                nc.tensor.matmul(
                                    ps[:, i, :],
                                    kp[hb:hb + 64, kt * P:(kt + 1) * P],
                                    q_sb[hb:hb + 64, hp, :],
                                    start=True, stop=True)
                            nc.scalar.activation(
                                e_sb[h][:, k0:k0 + klen, :], ps[:, 0:klen, :], AF.Exp)
                            # ctx accumulation: lhsT = [V_h(64) | ones(1)]
                            # contiguous -> psum rows 0:64 ctx, row 64 denom
                            for i in range(klen):
                                kt = k0 + i
                                vstart = kt * VKT + h * VHW
                                nc.tensor.matmul(
                                    ctx_ps[h][:], v_sb[:, vstart:vstart + VHW],
                                    e_sb[h][:, kt, :],
                                    start=(kt == 0), stop=(kt == NKT - 1))
                    for h in (2 * hp, 2 * hp + 1):
                        # ctx rows 0:64, denominator in row 64
                        r1 = rpool.tile([P, R], F16, tag="r1")
                        r128 = rpool.tile([P, R], F16, tag="r128")
                        with nc.allow_low_precision(reason="softmax recip to fp16"):
                            nc.vector.reciprocal(r1[64:65, :], ctx_ps[h][64:65, :])
                        nc.sync.dma_start(out=r1[0:1, :], in_=r1[64:65, :])
                        nc.gpsimd.partition_broadcast(r128[:], r1[0:1, :])
                        if h % 2 == 0:
                            nc.vector.tensor_mul(
                                ctx_sb[0:64, hp, :], ctx_ps[h][0:64, :], r128[0:64, :])
                        else:
                            ctmp = rpool.tile([P, R], F32R, tag="ctmp")
                            nc.vector.tensor_mul(
                                ctmp[0:64, :], ctx_ps[h][0:64, :], r128[0:64, :])
                            nc.sync.dma_start(
                                out=ctx_sb[64:128, hp, :], in_=ctmp[0:64, :])
                        # attn-weight mean accumulation (fp16):
                        #   wacc += e_h * r128   (first head: plain write)
                        acc = wacc if h % 2 == 0 else wacc2
                        for half in range(2):
                            sl = slice(half * 8, half * 8 + 8)
                            rb = _ap(r128, 0, [[R, P], [0, 8], [1, R]])
                            if h < 2:
                                nc.vector.tensor_mul(
                                    acc[:, sl, :], e_sb[h][:, sl, :], rb)
                            else:
                                wt = wtpool.tile([P, 8, R], F16, tag="wt",
                                                 bufs=1)
                                nc.vector.tensor_mul(wt[:], e_sb[h][:, sl, :], rb)
                                nc.vector.tensor_add(
                                    acc[:, sl, :], acc[:, sl, :], wt[:])

            for half in range(2):
                sl = slice(half * 8, half * 8 + 8)
                nc.vector.tensor_add(wacc[:, sl, :], wacc[:, sl, :],
                                     wacc2[:, sl, :])
            nc.sync.dma_start(
                out=wmeanT.rearrange("(kt p) q -> p kt q", p=P), in_=wacc[:])

            # ---------------- Phase 3: out_proj + residual + LayerNorm ------
            with tc.tile_pool(name="tail_sbuf", bufs=1) as tl, \
                 tc.tile_pool(name="tail_y", bufs=4) as ty, \
                 tc.tile_pool(name="tail_wo", bufs=4) as two, \
                 tc.tile_pool(name="tail_tmp", bufs=3) as tt, \
                 tc.tile_pool(name="tail_ps", bufs=1, space="PSUM") as tps:
                xr_sb = tl.tile([P, 4, D], F32)
                eps_sb = tl.tile([P, 1], F32)
                nc.vector.memset(eps_sb[:], LN_EPS)
                for rt in range(4):
                    nc.sync.dma_start(
                        out=xr_sb[:, rt, :], in_=xrow[rt * P:(rt + 1) * P, :])
                if has_gamma:
                    g_sb = tl.tile([P, D], F32)
                    nc.sync.dma_start(out=g_sb[:], in_=_ap(gamma, 0, [[0, P], [1, D]]))
                if has_beta:
                    be_sb = tl.tile([P, D], F32)
                    nc.sync.dma_start(out=be_sb[:], in_=_ap(beta, 0, [[0, P], [1, D]]))

                wo_sb = tl.tile([P, NDT, D], F32R)
                for jt in range(NDT):
                    nc.sync.dma_start(
                        out=wo_sb[:, jt, :], in_=woT[jt * P:(jt + 1) * P, :])
                ys = []
                for rt in range(4):
                    y = ty.tile([P, 2, 512], F32, tag="y", name=f"y{rt}")
                    ys.append(y)
                    pos = []
                    for dc in range(2):
                        po = tps.tile([P, 512], F32, tag="po", name=f"po{rt}_{dc}",
                                      bufs=4)
                        pos.append(po)
                    for jt in range(NDT):
                        for dc in range(2):
                            nc.tensor.matmul(
                                pos[dc][:], ctx_sb[:, jt, rt * P:(rt + 1) * P],
                                wo_sb[:, jt, dc * 512:(dc + 1) * 512],
                                start=(jt == 0),
                                stop=(jt == NDT - 1 and not has_out_bias))
                    for dc in range(2):
                        if has_out_bias:
                            nc.tensor.matmul(
                                pos[dc][:], ones_col[:],
                                b_sb[:, 3 * D + dc * 512:3 * D + (dc + 1) * 512],
                                start=False, stop=True)
                        nc.vector.tensor_add(
                            y[:, dc, :], pos[dc][:],
                            xr_sb[:, rt, dc * 512:(dc + 1) * 512])
                for rt in range(4):
                    y = ys[rt]
                    stats = tt.tile([P, 2, nc.vector.BN_STATS_DIM], F32, tag="st")
                    mv = tt.tile([P, nc.vector.BN_AGGR_DIM], F32, tag="mv")
                    for sg in range(2):
                        nc.vector.bn_stats(out=stats[:, sg, :], in_=y[:, sg, :])
                    nc.vector.bn_aggr(out=mv[:], in_=stats[:])
                    sd = tt.tile([P, 1], F32, tag="sd")
                    rstd = tt.tile([P, 1], F32, tag="rs")
                    nc.scalar.activation(sd[:], mv[:, 1:2], AF.Sqrt, bias=eps_sb[:])
                    nc.vector.reciprocal(rstd[:], sd[:])
                    yn = tt.tile([P, D], F32, tag="yn")
                    nc.vector.tensor_scalar(
                        out=yn[:], in0=_ap(y, 0, [[2 * 512, P], [1, D]]),
                        scalar1=mv[:, 0:1], scalar2=rstd[:],
                        op0=mybir.AluOpType.subtract, op1=mybir.AluOpType.mult)
                    if has_gamma:
                        nc.vector.tensor_mul(yn[:], yn[:], g_sb[:])
                    if has_beta:
                        nc.vector.tensor_add(yn[:], yn[:], be_sb[:])
                    nc.sync.dma_start(out=out_c[rt * P:(rt + 1) * P, :], in_=yn[:])

    nc.compile()
    return nc


_PROGRAM_CACHE = {}


def _get_program(flags):
    if flags not in _PROGRAM_CACHE:
        _PROGRAM_CACHE[flags] = build_program(*flags)
    return _PROGRAM_CACHE[flags]


def make_in_maps(x, in_proj_w, in_proj_b, out_proj_w, out_proj_b, ln_gamma, ln_beta):
    x = np.asarray(x, np.float32)
    in_proj_w = np.asarray(in_proj_w, np.float32)
    in_proj_b = np.asarray(in_proj_b, np.float32)
    out_proj_w = np.asarray(out_proj_w, np.float32)
    out_proj_b = np.asarray(out_proj_b, np.float32)
    ln_gamma = np.asarray(ln_gamma, np.float32)
    ln_beta = np.asarray(ln_beta, np.float32)

    scale = 1.0 / np.sqrt(HD)
    wqT = np.ascontiguousarray(in_proj_w[:D].T) * scale
    wkT = np.ascontiguousarray(in_proj_w[D:2 * D].T)
    wvT = np.ascontiguousarray(in_proj_w[2 * D:].T)
    woT = np.ascontiguousarray(out_proj_w.T)
    bvec = np.concatenate(
        [in_proj_b[:D] * scale, in_proj_b[D:], out_proj_b]).astype(np.float32)

    shared = dict(wqT=wqT, wkT=wkT, wvT=wvT, woT=woT, bvec=bvec,
                  gamma=ln_gamma, beta=ln_beta)
    in_maps = []
    for c in range(NCORES):
        b, sh = c // 4, c % 4
        xb = x[b]
        xbT = np.ascontiguousarray(xb.T)
        m = dict(shared)
        m["xT"] = xbT
        m["xQT"] = np.ascontiguousarray(xbT[:, sh * R:(sh + 1) * R])
        m["xrow"] = np.ascontiguousarray(xb[sh * R:(sh + 1) * R])
        in_maps.append(m)

    flags = (bool(in_proj_b.any()), bool(out_proj_b.any()),
             bool((ln_gamma != 1.0).any()), bool(ln_beta.any()))
    return in_maps, flags


def assemble(results):
    out = np.empty((B, S, D), np.float32)
    attn = np.empty((B, S, S), np.float32)
    for c in range(NCORES):
        b, sh = c // 4, c % 4
        out[b, sh * R:(sh + 1) * R] = results[c]["out_c"]
        attn[b, sh * R:(sh + 1) * R] = \
            results[c]["wmeanT"].astype(np.float32).T / H
    return out, attn


def kernel(x, in_proj_w, in_proj_b, out_proj_w, out_proj_b, ln_gamma, ln_beta):
    in_maps, flags = make_in_maps(
        x, in_proj_w, in_proj_b, out_proj_w, out_proj_b, ln_gamma, ln_beta)
    nc = _get_program(flags)
    res = run_bass_kernel_spmd(nc, in_maps, list(range(NCORES)))
    return assemble(res.results)
